# revision 24
# baseline (speedup 1.0000x reference)
"""MMoE layer kernel for 8 Trainium2 NeuronCores — mixed fp8/bf16 split-K.

Reference math (B=4096, D=1024, H1=2048, H2=1024, E=7 experts, NS=7 scenes):
  h        = relu(einsum('bd,edh', x, W1) + b1)           # [B,E,H1]
  eo       = relu(einsum('beh,eho', h, W2) + b2)          # [B,E,H2]
  xc       = concat(x, scene_emb[scene])                  # [B, D+16]
  G        = softmax over s of einsum('bd,sde', xc, S)    # [B,E,NS] (after transpose)
  q        = mean_s log(G*7)                              # [B,E]
  score1   = logG[b, e, scene_b]
  select   = drop expert e iff e == argmin_e score1 == argmin_e q
  gate     = softmax_e(G[b,e,scene_b]) * select
  out      = einsum('be,beo', gate, eo); output = stack([out, out])

Sharding: data-parallel over batch (512 rows/core), weights replicated.

Precision: fp8-e4m3 DoubleRow matmuls run at 2x the bf16 rate on TRN2, but
full-fp8 experts land at rel-err ~3.7e-2 (gate is 2e-2). So each layer's
contraction is SPLIT: the first FK k-tiles run in fp8 DoubleRow, the rest
in bf16, accumulating into the same PSUM group. Quantization noise scales
as sqrt(phi) (phi = fp8 fraction) while PE time drops by phi/2:
phi1=2/8, phi2=4/16 -> predicted rel ~1.9e-2, ~48us PE saved.
All operands are pre-scaled by powers of two (x*16, W1*128, h*16, W2*128)
in BOTH dtypes, so fp8 and bf16 partial sums share one PSUM scale; e4m3
operands sit in the normal range (no subnormal blowup). Scales divide back
out in the PSUM evacuations (L1: *SH/(SX*SW1); L2: gate pre-scaled by
1/(SH*SW2)). All routing math stays fp32 so the argmin/select decisions
are bit-stable.

Device decomposition of the routing (no cross-partition broadcasts):
  Gpre[b, e*7+s] = x[b] @ Sflat + SE_table[scene_b]   (SE_table = scene_emb @ S[:,D:,:])
  Z = sum_s exp(Gpre); logZ = ln Z; SG = sum_s Gpre
  q      = SG/7 - logZ            (+const, argmin only)
  score1 = sum_s Gpre*onehot_s(scene) - logZ
  gate0  = softmax_e(exp(score1)) (logits in (0,1): no max-subtract needed)
  sel    = 1 - ismin(score1)*ismin(q)
  gate   = gate0 * sel            (scaled by 1/(SH*SW2) for the L2 evac)

Startup: expert 0's layer 1 is emitted BEFORE the routing matmuls, and its
weights are DMA'd before xT/S, so the PE starts real work as soon as the
first weight lands instead of idling ~10us behind the routing stream.
"""

import sys

if "/opt/trn_rl_repo" not in sys.path:
    sys.path.insert(0, "/opt/trn_rl_repo")

from contextlib import ExitStack

import ml_dtypes
import numpy as np

import concourse.bass as bass
import concourse.tile as tile
from concourse import bacc, mybir
from concourse.bass_utils import run_bass_kernel_spmd

F32 = mybir.dt.float32
BF16 = mybir.dt.bfloat16
FP8 = mybir.dt.float8e4
AF = mybir.ActivationFunctionType
ALU = mybir.AluOpType
AX = mybir.AxisListType
DR = mybir.MatmulPerfMode.DoubleRow

N_CORES = 8
B, D, H1, H2, E, NS, T = 4096, 1024, 2048, 1024, 7, 7, 2
BL = B // N_CORES          # 512 rows per core
NB = BL // 128             # 4 batch tiles
KT1 = D // 128             # 8  k-tiles, layer 1
MT1 = H1 // 128            # 16 m-tiles, layer 1
KT2 = H1 // 128            # 16 k-tiles, layer 2
NO = H2 // 512             # 2  512-wide out column blocks
EN = E * NS                # 49
NP_FP8 = np.dtype(ml_dtypes.float8_e4m3)
NP_BF16 = np.dtype(ml_dtypes.bfloat16)

FK1 = 2                    # fp8 k-tiles in layer 1 (of KT1)
FK2 = 4                    # fp8 k-tiles in layer 2 (of KT2)
BK1 = KT1 - FK1            # bf16 k-tiles, layer 1
BK2 = KT2 - FK2            # bf16 k-tiles, layer 2

# power-of-two scales keeping e4m3 operands in the normal range
SX = 16.0      # x * SX      (x ~ N(0,1))
SW1 = 128.0    # W1 * SW1    (W1 ~ N(0, 1/1024))
SH = 16.0      # h * SH      (h = relu(~N(0,1)))
SW2 = 128.0    # W2 * SW2    (W2 ~ N(0, 1/2048))
L1_EVAC_SCALE = SH / (SX * SW1)      # psum*this = h*SH
GATE_SCALE = 1.0 / (SH * SW2)        # folded into the gate tile


def _emit_kernel(tc, aps, has_b1, has_b2):
    nc = tc.nc
    ctx = ExitStack()
    with ctx:
        # Pool stack order matters: the expert-weight pools are allocated
        # BEFORE the routing pool so they never reuse the routing pool's
        # released SBUF addresses — otherwise Tile serializes the first
        # weight DMAs behind every routing matmul (measured 13µs PE stall).
        consts = ctx.enter_context(tc.tile_pool(name="consts", bufs=1))
        w1pool = ctx.enter_context(tc.tile_pool(name="w1", bufs=2))
        w2pool = ctx.enter_context(tc.tile_pool(name="w2", bufs=2))
        htpool = ctx.enter_context(tc.tile_pool(name="ht", bufs=1))
        tmppool = ctx.enter_context(tc.tile_pool(name="tmp", bufs=3))
        l1ps = ctx.enter_context(tc.tile_pool(name="l1ps", bufs=4, space="PSUM"))
        l2ps = ctx.enter_context(tc.tile_pool(name="l2ps", bufs=4, space="PSUM"))
        rpool = tc.alloc_tile_pool(name="routing", bufs=1)

        # ---- PE warm-up: dummy matmuls from memset tiles (no input deps)
        # fill the ~10µs DMA boot window and flip the HAM clock gate to 8/8
        # before the real matmuls run. ------------------------------------
        warm_sb = rpool.tile([128, 512], BF16)
        nc.vector.memset(warm_sb[:, :], 0.0)
        warm_ps = l1ps.tile([128, 512], F32, tag="ps1", name="warm_ps")
        for _ in range(9):
            nc.tensor.matmul(
                warm_ps[:, :], lhsT=warm_sb[:, 0:128], rhs=warm_sb[:, :],
                start=True, stop=True,
            )

        # ---- layer-1 inputs lead the sync queue so expert 0 starts as
        # early as possible; routing inputs (xT fp32 + S tables) follow.
        # x8 + w1f[0] (0.625MB) lead everything: expert 0's DoubleRow
        # prelude needs only those two and starts ~13µs in, inside the
        # window the warmup used to burn. ---------------------------------
        sflat_sb = rpool.tile([128, KT1, EN], F32)
        xt8_sb = consts.tile([128, FK1, BL], FP8)
        nc.sync.dma_start(xt8_sb[:, :, :], aps["xT8"].rearrange("(t p) b -> p t b", p=128))
        w1f0_sb = w1pool.tile([128, FK1, H1], FP8, tag="w1f")
        nc.sync.dma_start(
            w1f0_sb[:, :, :], aps["w1f"][0].rearrange("(t p) h -> p t h", p=128)
        )
        xtb_sb = consts.tile([128, BK1, BL], BF16)
        nc.sync.dma_start(xtb_sb[:, :, :], aps["xTb"].rearrange("(t p) b -> p t b", p=128))

        gate_sb = consts.tile([128, NB, E], F32)
        acc_sb = consts.tile([128, NB, H2], F32)
        if has_b1:
            b1_sb = consts.tile([128, E * MT1], F32)
            nc.sync.dma_start(b1_sb[:, :], aps["b1t"][:, :])
        if has_b2:
            b2_sb = consts.tile([1, E * H2], BF16)
            nc.sync.dma_start(b2_sb[:, :], aps["b2f"][:, :])
            ones_sb = consts.tile([1, 128], BF16)
            nc.vector.memset(ones_sb[:, :], 1.0)

        def dma_weights(e):
            """Queue expert e's weights; returns (w1f, w1b, w2f, w2b) tiles.

            All bulk traffic rides the sync HWDGE queue in program order —
            the per-core DMA fabric saturates at ~350GB/s regardless of
            queue count, so ordering (not parallel queues) is what matters.
            Two half-DMAs per bf16 weight: one trigger splits across all 16
            SDMA engines, and halves complete earlier than one monolithic
            semaphore.
            """
            w1f_sb = w1pool.tile([128, FK1, H1], FP8, tag="w1f")
            nc.sync.dma_start(
                w1f_sb[:, :, :], aps["w1f"][e].rearrange("(t p) h -> p t h", p=128)
            )
            w1b_sb = w1pool.tile([128, BK1, H1], BF16, tag="w1b")
            w1b_src = aps["w1b"][e].rearrange("(t p) h -> p t h", p=128)
            # Column-split halves: layer 1's m-loop consumes columns in
            # order, so m-tiles 0-7 start as soon as the first half lands.
            nc.sync.dma_start(w1b_sb[:, :, 0 : H1 // 2], w1b_src[:, :, 0 : H1 // 2])
            nc.sync.dma_start(w1b_sb[:, :, H1 // 2 :], w1b_src[:, :, H1 // 2 :])
            w2f_sb = w2pool.tile([128, FK2, H2], FP8, tag="w2f")
            nc.sync.dma_start(
                w2f_sb[:, :, :], aps["w2f"][e].rearrange("(t p) o -> p t o", p=128)
            )
            w2b_sb = w2pool.tile([128, BK2, H2], BF16, tag="w2b")
            w2b_src = aps["w2b"][e].rearrange("(t p) o -> p t o", p=128)
            # Column-split halves: the (mb, no=0) PSUM groups only need the
            # first output-column half, so layer 2 can start 1.5MB earlier.
            nc.sync.dma_start(w2b_sb[:, :, 0 : H2 // 2], w2b_src[:, :, 0 : H2 // 2])
            nc.sync.dma_start(w2b_sb[:, :, H2 // 2 :], w2b_src[:, :, H2 // 2 :])
            return w1f_sb, w1b_sb, w2f_sb, w2b_sb

        def layer1(e, w1f_sb, w1b_sb, interleave=None, hpart=None):
            """hT[f, b] = relu(sum_d W1[d, f]*x[b, d] + b1[f]), mixed fp8/bf16.

            Returns (ht8, htb): m-tiles < FK2 evacuate to fp8 (they are
            layer 2's fp8 k-range), the rest to bf16.

            interleave: {m: emit_fn} — expert 0 slips the routing matmuls
            between its late m-groups, keeping the PE stream dense so the
            49-wide fp32 matmuls never let HAM clock-gate the array, and
            the gate is ready right when layer 2's evacuations want it.

            hpart: expert 0's DoubleRow partials already computed in the
            boot window; this loop then runs only the bf16 k-tiles and the
            evacuation adds the partial back in.
            """
            ht8_sb = htpool.tile([128, FK2, BL], FP8, tag="ht8")
            htb_sb = htpool.tile([128, BK2, BL], BF16, tag="htb")
            for m in range(MT1):
                if interleave and m in interleave:
                    interleave[m]()
                ps = l1ps.tile([128, BL], F32, tag="ps1")
                if hpart is None:
                    for j in range(FK1 // 2):
                        nc.tensor.matmul(
                            ps[:, :],
                            lhsT=w1f_sb[:, 2 * j : 2 * j + 2, bass.ts(m, 128)],
                            rhs=xt8_sb[:, 2 * j : 2 * j + 2, :],
                            start=(j == 0), stop=False,
                            perf_mode=DR, skip_group_check=True,
                        )
                for kt in range(BK1):
                    nc.tensor.matmul(
                        ps[:, :],
                        lhsT=w1b_sb[:, kt, bass.ts(m, 128)],
                        rhs=xtb_sb[:, kt, :],
                        start=(hpart is not None and kt == 0),
                        stop=(kt == BK1 - 1),
                        skip_group_check=True,
                    )
                dst = ht8_sb[:, m, :] if m < FK2 else htb_sb[:, m - FK2, :]
                bias1 = b1_sb[:, e * MT1 + m : e * MT1 + m + 1] if has_b1 else 0.0
                if hpart is not None:
                    # ps holds only the bf16 k-range: add the DR partial,
                    # then relu+scale+cast in a second pass.
                    tsum = tmppool.tile([128, BL], F32, tag="tmp")
                    nc.vector.tensor_tensor(
                        out=tsum[:, :], in0=ps[:, :], in1=hpart[:, m, :], op=ALU.add
                    )
                    src = tsum
                else:
                    src = ps
                # Evacuations alternate scalar/vector so neither engine's
                # FIFO becomes the PSUM-slot bottleneck.
                # (The vector path has no bias input, so b1 forces scalar.)
                if has_b1 or m % 2 == 0:
                    nc.scalar.activation(
                        dst, src[:, :], AF.Relu, bias=bias1, scale=L1_EVAC_SCALE
                    )
                else:
                    nc.vector.tensor_scalar(
                        out=dst, in0=src[:, :],
                        scalar1=L1_EVAC_SCALE, scalar2=0.0,
                        op0=ALU.mult, op1=ALU.max,
                    )
            return ht8_sb, htb_sb

        def layer2(e, ht8_sb, htb_sb, w2f_sb, w2b_sb):
            """out[b, o] += gate[b, e]*relu(sum_h hT[h, b]*W2[h, o] + b2[o])."""
            for mb in range(NB):
                for no in range(NO):
                    ps2 = l2ps.tile([128, 512], F32, tag="ps2")
                    for j in range(FK2 // 2):
                        nc.tensor.matmul(
                            ps2[:, :],
                            lhsT=ht8_sb[:, 2 * j : 2 * j + 2, bass.ts(mb, 128)],
                            rhs=w2f_sb[:, 2 * j : 2 * j + 2, bass.ts(no, 512)],
                            start=(j == 0), stop=False,
                            perf_mode=DR, skip_group_check=True,
                        )
                    for kt in range(BK2):
                        nc.tensor.matmul(
                            ps2[:, :],
                            lhsT=htb_sb[:, kt, bass.ts(mb, 128)],
                            rhs=w2b_sb[:, kt, bass.ts(no, 512)],
                            start=False,
                            stop=(kt == BK2 - 1 and not has_b2),
                            skip_group_check=True,
                        )
                    if has_b2:
                        nc.tensor.matmul(
                            ps2[:, :],
                            lhsT=ones_sb[:, :],
                            rhs=b2_sb[:, e * H2 + no * 512 : e * H2 + (no + 1) * 512],
                            start=False, stop=True, skip_group_check=True,
                        )
                    gcol = gate_sb[:, mb, e : e + 1]
                    if e == 0:
                        nc.scalar.activation(
                            acc_sb[:, mb, bass.ts(no, 512)], ps2[:, :], AF.Relu, scale=gcol
                        )
                    elif e < E - 1:
                        tmp = tmppool.tile([128, 512], F32, tag="tmp")
                        nc.scalar.activation(tmp[:, :], ps2[:, :], AF.Relu, scale=gcol)
                        nc.vector.tensor_tensor(
                            out=acc_sb[:, mb, bass.ts(no, 512)],
                            in0=acc_sb[:, mb, bass.ts(no, 512)],
                            in1=tmp[:, :], op=ALU.add,
                        )
                    else:
                        # Last expert: 256-column chunks so the evac -> add ->
                        # store chain pipelines and the kernel tail shrinks.
                        tmp = tmppool.tile([128, 512], F32, tag="tmp")
                        out_dst = aps["out"].rearrange("(t p) o -> p t o", p=128)
                        for hh in range(2):
                            c0 = no * 512 + hh * 256
                            nc.scalar.activation(
                                tmp[:, hh * 256 : (hh + 1) * 256],
                                ps2[:, hh * 256 : (hh + 1) * 256],
                                AF.Relu, scale=gcol,
                            )
                            nc.vector.tensor_tensor(
                                out=acc_sb[:, mb, c0 : c0 + 256],
                                in0=acc_sb[:, mb, c0 : c0 + 256],
                                in1=tmp[:, hh * 256 : (hh + 1) * 256], op=ALU.add,
                            )
                            nc.sync.dma_start(
                                out_dst[:, mb, c0 : c0 + 256],
                                acc_sb[:, mb, c0 : c0 + 256],
                            )

        # ---- expert 0 layer 1 first: the PE's first real work only needs
        # x + W1[0], which lead the DMA queue. -----------------------------
        # Expert 0's bf16 W1 streams in four column-quarters: layer 1's
        # m-tiles 0-3 only need the first 0.75MB, so the PE starts ~2µs
        # earlier than with halves. sflat (routing, consumed at m>=10)
        # rides after the second quarter.
        w1b0_sb = w1pool.tile([128, BK1, H1], BF16, tag="w1b")
        w1b0_src = aps["w1b"][0].rearrange("(t p) h -> p t h", p=128)
        for qq in range(2):
            nc.sync.dma_start(
                w1b0_sb[:, :, qq * 512 : (qq + 1) * 512],
                w1b0_src[:, :, qq * 512 : (qq + 1) * 512],
            )
        nc.sync.dma_start(sflat_sb[:, :, :], aps["sflat"].rearrange("(t p) j -> p t j", p=128))
        for qq in range(2, 4):
            nc.sync.dma_start(
                w1b0_sb[:, :, qq * 512 : (qq + 1) * 512],
                w1b0_src[:, :, qq * 512 : (qq + 1) * 512],
            )

        # ---- routing inputs: xT quarters (kt-major matmuls consume k-slices
        # as they land), then the small tables, then expert 0's W2. --------
        xt_sb = rpool.tile([128, KT1, BL], F32)
        xt_src = aps["xT"].rearrange("(t p) b -> p t b", p=128)
        for qq in range(4):
            nc.sync.dma_start(
                xt_sb[:, 2 * qq : 2 * qq + 2, :], xt_src[:, 2 * qq : 2 * qq + 2, :]
            )
        sett_sb = rpool.tile([10, EN], F32)
        nc.sync.dma_start(sett_sb[:, :], aps["sett"][:, :])
        scolr_sb = rpool.tile([128, NB * EN], F32)
        nc.sync.dma_start(scolr_sb[:, :], aps["scol_rep"][:, :])
        srow10_sb = rpool.tile([10, BL], F32)
        nc.sync.dma_start(srow10_sb[:, :], aps["srow"].to_broadcast((10, BL)))
        io7_sb = rpool.tile([128, NB * EN], F32)
        nc.sync.dma_start(io7_sb[:, :], aps["iota7"].to_broadcast((128, NB * EN)))
        io10_sb = rpool.tile([10, 1], F32)
        nc.sync.dma_start(io10_sb[:, :], aps["iota10"][:, :])

        w2f0_sb = w2pool.tile([128, FK2, H2], FP8, tag="w2f")
        nc.sync.dma_start(
            w2f0_sb[:, :, :], aps["w2f"][0].rearrange("(t p) o -> p t o", p=128)
        )
        w2b0_sb = w2pool.tile([128, BK2, H2], BF16, tag="w2b")
        w2b0_src = aps["w2b"][0].rearrange("(t p) o -> p t o", p=128)
        nc.sync.dma_start(w2b0_sb[:, :, 0 : H2 // 2], w2b0_src[:, :, 0 : H2 // 2])
        nc.sync.dma_start(w2b0_sb[:, :, H2 // 2 :], w2b0_src[:, :, H2 // 2 :])

        # onehot over embedding rows, [10, BL]: onehot[r, b] = (scene[b] == r).
        # On GPSIMD (otherwise idle): the DVE FIFO is full of layer-1
        # evacuations, which would delay this until ~the routing tail and
        # leave the PE's onehot matmuls briefly starved (measured 6.8µs
        # half-clock dip at the L2(0) transition).
        onehot_sb = rpool.tile([10, BL], F32)
        nc.gpsimd.tensor_scalar(
            out=onehot_sb[:, :], in0=srow10_sb[:, :],
            scalar1=io10_sb[:, 0:1], scalar2=None, op0=ALU.is_equal,
        )

        # Routing matmuls slip between expert 0's late L1 m-groups (the xT
        # quarters have landed by then), keeping the PE stream dense so the
        # 49-wide fp32 matmuls never let HAM clock-gate the array, and the
        # gate is ready right when layer 2's evacuations want it. The psr
        # PSUM groups live in the L2 pool: its first real tiles are only
        # allocated after the gp copies release these, so the rotation
        # cannot deadlock against L1's.
        psr = []
        for t in range(NB):
            psr_t = l2ps.tile([128, EN], F32, tag="ps2", name=f"psr{t}")
            psr.append(psr_t)
        gp = rpool.tile([128, NB * EN], F32)  # all 4 b-tiles side by side

        def routing_chunk(kts):
            def emit():
                for kt in kts:
                    for t in range(NB):
                        nc.tensor.matmul(
                            psr[t][:, :],
                            lhsT=xt_sb[:, kt, bass.ts(t, 128)],
                            rhs=sflat_sb[:, kt, :],
                            start=(kt == 0), stop=False,
                        )
            return emit

        def routing_finish():
            for t in range(NB):
                nc.tensor.matmul(
                    psr[t][:, :],
                    lhsT=onehot_sb[:, bass.ts(t, 128)],
                    rhs=sett_sb[:, :],
                    start=False, stop=True,
                )

        # DoubleRow prelude: expert 0's 16 fp8 matmuls depend only on x8 +
        # w1f[0] (first 0.625MB of the stream), so they run ~13µs in —
        # inside the window the warmup used to burn — and their partials
        # park in SBUF until the bf16 k-tiles catch up.
        hpart0 = htpool.tile([128, MT1, BL], BF16, tag="hpart")
        for m in range(MT1):
            psp = l1ps.tile([128, BL], F32, tag="ps1")
            nc.tensor.matmul(
                psp[:, :],
                lhsT=w1f0_sb[:, 0:2, bass.ts(m, 128)],
                rhs=xt8_sb[:, 0:2, :],
                start=True, stop=True, perf_mode=DR, skip_group_check=True,
            )
            if m % 2 == 0:
                nc.scalar.copy(hpart0[:, m, :], psp[:, :])
            else:
                nc.vector.tensor_scalar(
                    out=hpart0[:, m, :], in0=psp[:, :],
                    scalar1=1.0, scalar2=None, op0=ALU.mult,
                )
        # bridge warmup: keep the PE dense until w1b[0]'s first quarter
        warm_ps2 = l1ps.tile([128, 512], F32, tag="ps1", name="warm_ps2")
        for _ in range(6):
            nc.tensor.matmul(
                warm_ps2[:, :], lhsT=warm_sb[:, 0:128], rhs=warm_sb[:, :],
                start=True, stop=True,
            )

        ht8_0, htb_0 = layer1(
            0, w1f0_sb, w1b0_sb,
            interleave={
                10: routing_chunk([0, 1]),
                11: routing_chunk([2, 3]),
                12: routing_chunk([4, 5]),
                13: routing_chunk([6, 7]),
                14: routing_finish,
            },
            hpart=hpart0,
        )
        for t in range(NB):
            nc.scalar.copy(gp[:, bass.ts(t, EN)], psr[t][:, :])

        def routing_chain():
            """Gate computation, fused over all 4 b-tiles ([128, 4*49]).

            Emitted AFTER layer 1 of expert 0: the scalar engine's queue is
            strict FIFO, so emitting this serial chain before the L1 PSUM
            evacuations would block them (and stall the PE on PSUM slots).
            The gate is only consumed by expert 0's layer-2 evacuation.
            """
            NE = NB * E  # 28
            gp4 = gp.rearrange("p (t e s) -> p (t e) s", s=NS, e=E)
            eex = rpool.tile([128, NB * EN], F32)
            nc.scalar.activation(eex[:, :], gp[:, :], AF.Exp)
            z = rpool.tile([128, NE], F32)
            nc.vector.tensor_reduce(out=z[:, :], in_=eex.rearrange("p (t e s) -> p (t e) s", s=NS, e=E), axis=AX.X, op=ALU.add)
            logz = rpool.tile([128, NE], F32)
            nc.scalar.activation(logz[:, :], z[:, :], AF.Ln)
            sg = rpool.tile([128, NE], F32)
            nc.vector.tensor_reduce(out=sg[:, :], in_=gp4, axis=AX.X, op=ALU.add)
            q = rpool.tile([128, NE], F32)
            nc.vector.scalar_tensor_tensor(
                out=q[:, :], in0=sg[:, :], scalar=1.0 / NS, in1=logz[:, :],
                op0=ALU.mult, op1=ALU.subtract,
            )
            oh = rpool.tile([128, NB * EN], F32)
            nc.vector.tensor_tensor(out=oh[:, :], in0=io7_sb[:, :], in1=scolr_sb[:, :], op=ALU.is_equal)
            gsel = rpool.tile([128, NB * EN], F32)
            nc.vector.tensor_tensor(out=gsel[:, :], in0=gp[:, :], in1=oh[:, :], op=ALU.mult)
            s1s = rpool.tile([128, NE], F32)
            nc.vector.tensor_reduce(out=s1s[:, :], in_=gsel.rearrange("p (t e s) -> p (t e) s", s=NS, e=E), axis=AX.X, op=ALU.add)
            score1 = rpool.tile([128, NE], F32)
            nc.vector.tensor_tensor(out=score1[:, :], in0=s1s[:, :], in1=logz[:, :], op=ALU.subtract)

            lg = rpool.tile([128, NE], F32)
            nc.scalar.activation(lg[:, :], score1[:, :], AF.Exp)     # G at scene, in (0,1)
            el = rpool.tile([128, NE], F32)
            nc.scalar.activation(el[:, :], lg[:, :], AF.Exp)         # softmax numerator
            # per-b-tile scalars ([128,1]) for the reductions' broadcasts
            ssum = rpool.tile([128, NB], F32)
            rs = rpool.tile([128, NB], F32)
            m1 = rpool.tile([128, NB], F32)
            m2 = rpool.tile([128, NB], F32)
            k1 = rpool.tile([128, NE], F32)
            k2 = rpool.tile([128, NE], F32)
            g0 = rpool.tile([128, NE], F32)
            el3 = el.rearrange("p (t e) -> p t e", e=E)
            sc3 = score1.rearrange("p (t e) -> p t e", e=E)
            q3 = q.rearrange("p (t e) -> p t e", e=E)
            nc.vector.tensor_reduce(out=ssum[:, :], in_=el3, axis=AX.X, op=ALU.add)
            nc.vector.reciprocal(rs[:, :], ssum[:, :])
            nc.vector.tensor_reduce(out=m1[:, :], in_=sc3, axis=AX.X, op=ALU.min)
            nc.vector.tensor_reduce(out=m2[:, :], in_=q3, axis=AX.X, op=ALU.min)
            for t in range(NB):
                nc.vector.tensor_scalar(
                    out=k1[:, bass.ts(t, E)], in0=score1[:, bass.ts(t, E)],
                    scalar1=m1[:, t : t + 1], scalar2=None, op0=ALU.is_equal,
                )
                nc.vector.tensor_scalar(
                    out=k2[:, bass.ts(t, E)], in0=q[:, bass.ts(t, E)],
                    scalar1=m2[:, t : t + 1], scalar2=None, op0=ALU.is_equal,
                )
                nc.vector.tensor_scalar(
                    out=g0[:, bass.ts(t, E)], in0=el[:, bass.ts(t, E)],
                    scalar1=rs[:, t : t + 1], scalar2=None, op0=ALU.mult,
                )
            kill = rpool.tile([128, NE], F32)
            nc.vector.tensor_tensor(out=kill[:, :], in0=k1[:, :], in1=k2[:, :], op=ALU.mult)
            sel = rpool.tile([128, NE], F32)
            nc.vector.tensor_scalar(
                out=sel[:, :], in0=kill[:, :], scalar1=-1.0, scalar2=1.0,
                op0=ALU.mult, op1=ALU.add,
            )
            gate_flat = gate_sb.rearrange("p t e -> p (t e)")
            # gate = g0 * sel, pre-scaled by 1/(SH*SW2) for the L2 evacuation
            nc.vector.scalar_tensor_tensor(
                out=gate_flat[:, :], in0=g0[:, :], scalar=GATE_SCALE,
                in1=sel[:, :], op0=ALU.mult, op1=ALU.mult,
            )

        routing_chain()
        rpool.release()

        # ---- expert 0 layer 2, then experts 1..6 -------------------------
        layer2(0, ht8_0, htb_0, w2f0_sb, w2b0_sb)
        for e in range(1, E):
            w1f_sb, w1b_sb, w2f_sb, w2b_sb = dma_weights(e)
            ht8_sb, htb_sb = layer1(e, w1f_sb, w1b_sb)
            layer2(e, ht8_sb, htb_sb, w2f_sb, w2b_sb)


def build(has_b1, has_b2):
    """Build + schedule + compile the Bass program. Returns nc."""
    nc = bacc.Bacc("TRN2", target_bir_lowering=False, debug=False)
    aps = {}
    aps["xT"] = nc.dram_tensor("xT", [D, BL], F32, kind="ExternalInput").ap()
    aps["xT8"] = nc.dram_tensor("xT8", [FK1 * 128, BL], FP8, kind="ExternalInput").ap()
    aps["xTb"] = nc.dram_tensor("xTb", [BK1 * 128, BL], BF16, kind="ExternalInput").ap()
    aps["w1f"] = nc.dram_tensor("w1f", [E, FK1 * 128, H1], FP8, kind="ExternalInput").ap()
    aps["w1b"] = nc.dram_tensor("w1b", [E, BK1 * 128, H1], BF16, kind="ExternalInput").ap()
    aps["w2f"] = nc.dram_tensor("w2f", [E, FK2 * 128, H2], FP8, kind="ExternalInput").ap()
    aps["w2b"] = nc.dram_tensor("w2b", [E, BK2 * 128, H2], BF16, kind="ExternalInput").ap()
    if has_b1:
        aps["b1t"] = nc.dram_tensor("b1t", [128, E * MT1], F32, kind="ExternalInput").ap()
    if has_b2:
        aps["b2f"] = nc.dram_tensor("b2f", [1, E * H2], BF16, kind="ExternalInput").ap()
    aps["sflat"] = nc.dram_tensor("sflat", [D, EN], F32, kind="ExternalInput").ap()
    aps["sett"] = nc.dram_tensor("sett", [10, EN], F32, kind="ExternalInput").ap()
    aps["scol_rep"] = nc.dram_tensor("scol_rep", [128, NB * EN], F32, kind="ExternalInput").ap()
    aps["srow"] = nc.dram_tensor("srow", [1, BL], F32, kind="ExternalInput").ap()
    aps["iota7"] = nc.dram_tensor("iota7", [1, NB * EN], F32, kind="ExternalInput").ap()
    aps["iota10"] = nc.dram_tensor("iota10", [10, 1], F32, kind="ExternalInput").ap()
    aps["out"] = nc.dram_tensor("out", [BL, H2], F32, kind="ExternalOutput").ap()

    with tile.TileContext(nc) as tc:
        _emit_kernel(tc, aps, has_b1, has_b2)
    nc.compile()
    return nc


def make_in_maps(inputs):
    """Host-side layout prep + batch sharding. Returns (in_maps, has_b1, has_b2)."""
    x = np.ascontiguousarray(np.asarray(inputs["x"], dtype=np.float32))
    scene = np.asarray(inputs["scene"]).astype(np.int64)
    W1 = np.asarray(inputs["W1"], dtype=np.float32)
    b1 = np.asarray(inputs["b1"], dtype=np.float32)
    W2 = np.asarray(inputs["W2"], dtype=np.float32)
    b2 = np.asarray(inputs["b2"], dtype=np.float32)
    S = np.asarray(inputs["S"], dtype=np.float32)
    scene_emb = np.asarray(inputs["scene_emb"], dtype=np.float32)

    has_b1 = bool(np.any(b1))
    has_b2 = bool(np.any(b2))

    d_f1 = FK1 * 128           # fp8 rows of the L1 contraction
    h_f2 = FK2 * 128           # fp8 rows of the L2 contraction
    w1s = W1 * SW1
    w2s = W2 * SW2
    shared = {
        "w1f": np.ascontiguousarray(w1s[:, :d_f1, :].astype(NP_FP8)),
        "w1b": np.ascontiguousarray(w1s[:, d_f1:, :].astype(NP_BF16)),
        "w2f": np.ascontiguousarray(w2s[:, :h_f2, :].astype(NP_FP8)),
        "w2b": np.ascontiguousarray(w2s[:, h_f2:, :].astype(NP_BF16)),
        "sflat": np.ascontiguousarray(S[:, :D, :].transpose(1, 2, 0).reshape(D, EN)),
        "sett": np.ascontiguousarray(
            np.einsum("rm,sme->res", scene_emb, S[:, D:, :]).reshape(scene_emb.shape[0], EN)
        ),
        "iota7": np.tile(np.arange(EN, dtype=np.float32) % NS, NB).reshape(1, NB * EN),
        "iota10": np.arange(10, dtype=np.float32).reshape(10, 1),
    }
    if has_b1:
        shared["b1t"] = np.ascontiguousarray(
            b1.reshape(E, MT1, 128).transpose(2, 0, 1).reshape(128, E * MT1)
            * SH  # activation computes relu(psum*scale + bias); bias = b1*SH
        )
    if has_b2:
        shared["b2f"] = np.ascontiguousarray(
            (b2 * SH * SW2).astype(NP_BF16).reshape(1, E * H2)
        )

    in_maps = []
    for c in range(N_CORES):
        xs = x[c * BL : (c + 1) * BL]
        sc = scene[c * BL : (c + 1) * BL]
        xT = np.ascontiguousarray(xs.T)
        xTs = xT * SX
        m = dict(shared)
        m["xT"] = xT
        m["xT8"] = np.ascontiguousarray(xTs[:d_f1].astype(NP_FP8))
        m["xTb"] = np.ascontiguousarray(xTs[d_f1:].astype(NP_BF16))
        scol = sc.reshape(NB, 128).T.astype(np.float32)          # [128, NB]
        m["scol_rep"] = np.ascontiguousarray(
            np.repeat(scol[:, :, None], EN, axis=2).reshape(128, NB * EN)
        )
        m["srow"] = np.ascontiguousarray(sc.astype(np.float32).reshape(1, BL))
        in_maps.append(m)
    return in_maps, has_b1, has_b2


_NC_CACHE = {}


def get_compiled(has_b1, has_b2):
    key = (has_b1, has_b2)
    if key not in _NC_CACHE:
        _NC_CACHE[key] = build(has_b1, has_b2)
    return _NC_CACHE[key]


def run(inputs, trace=False, **kwargs):
    """Run on hardware; returns (full_output, BassKernelResults)."""
    in_maps, has_b1, has_b2 = make_in_maps(inputs)
    nc = get_compiled(has_b1, has_b2)
    res = run_bass_kernel_spmd(nc, in_maps, core_ids=list(range(N_CORES)), trace=trace, **kwargs)
    parts = [res.results[c]["out"] for c in range(N_CORES)]
    out = np.concatenate(parts, axis=0).astype(np.float32)
    full = np.ascontiguousarray(np.broadcast_to(out[None], (T, B, H2)))
    return full, res


def kernel(**inputs):
    full, _ = run(inputs, trace=False)
    return full


# revision 25
# speedup vs baseline: 1.0185x; 1.0185x over previous
"""MMoE layer kernel for 8 Trainium2 NeuronCores — mixed fp8/bf16 split-K.

Reference math (B=4096, D=1024, H1=2048, H2=1024, E=7 experts, NS=7 scenes):
  h        = relu(einsum('bd,edh', x, W1) + b1)           # [B,E,H1]
  eo       = relu(einsum('beh,eho', h, W2) + b2)          # [B,E,H2]
  xc       = concat(x, scene_emb[scene])                  # [B, D+16]
  G        = softmax over s of einsum('bd,sde', xc, S)    # [B,E,NS] (after transpose)
  q        = mean_s log(G*7)                              # [B,E]
  score1   = logG[b, e, scene_b]
  select   = drop expert e iff e == argmin_e score1 == argmin_e q
  gate     = softmax_e(G[b,e,scene_b]) * select
  out      = einsum('be,beo', gate, eo); output = stack([out, out])

Sharding: data-parallel over batch (512 rows/core), weights replicated.

Precision: fp8-e4m3 DoubleRow matmuls run at 2x the bf16 rate on TRN2, but
full-fp8 experts land at rel-err ~3.7e-2 (gate is 2e-2). So each layer's
contraction is SPLIT: the first FK k-tiles run in fp8 DoubleRow, the rest
in bf16, accumulating into the same PSUM group. Quantization noise scales
as sqrt(phi) (phi = fp8 fraction) while PE time drops by phi/2:
phi1=2/8, phi2=4/16 -> predicted rel ~1.9e-2, ~48us PE saved.
All operands are pre-scaled by powers of two (x*16, W1*128, h*16, W2*128)
in BOTH dtypes, so fp8 and bf16 partial sums share one PSUM scale; e4m3
operands sit in the normal range (no subnormal blowup). Scales divide back
out in the PSUM evacuations (L1: *SH/(SX*SW1); L2: gate pre-scaled by
1/(SH*SW2)). All routing math stays fp32 so the argmin/select decisions
are bit-stable.

Device decomposition of the routing (no cross-partition broadcasts):
  Gpre[b, e*7+s] = x[b] @ Sflat + SE_table[scene_b]   (SE_table = scene_emb @ S[:,D:,:])
  Z = sum_s exp(Gpre); logZ = ln Z; SG = sum_s Gpre
  q      = SG/7 - logZ            (+const, argmin only)
  score1 = sum_s Gpre*onehot_s(scene) - logZ
  gate0  = softmax_e(exp(score1)) (logits in (0,1): no max-subtract needed)
  sel    = 1 - ismin(score1)*ismin(q)
  gate   = gate0 * sel            (scaled by 1/(SH*SW2) for the L2 evac)

Startup: expert 0's layer 1 is emitted BEFORE the routing matmuls, and its
weights are DMA'd before xT/S, so the PE starts real work as soon as the
first weight lands instead of idling ~10us behind the routing stream.
"""

import sys

if "/opt/trn_rl_repo" not in sys.path:
    sys.path.insert(0, "/opt/trn_rl_repo")

from contextlib import ExitStack

import ml_dtypes
import numpy as np

import concourse.bass as bass
import concourse.tile as tile
from concourse import bacc, mybir
from concourse.bass_utils import run_bass_kernel_spmd

F32 = mybir.dt.float32
BF16 = mybir.dt.bfloat16
FP8 = mybir.dt.float8e4
AF = mybir.ActivationFunctionType
ALU = mybir.AluOpType
AX = mybir.AxisListType
DR = mybir.MatmulPerfMode.DoubleRow

N_CORES = 8
B, D, H1, H2, E, NS, T = 4096, 1024, 2048, 1024, 7, 7, 2
BL = B // N_CORES          # 512 rows per core
NB = BL // 128             # 4 batch tiles
KT1 = D // 128             # 8  k-tiles, layer 1
MT1 = H1 // 128            # 16 m-tiles, layer 1
KT2 = H1 // 128            # 16 k-tiles, layer 2
NO = H2 // 512             # 2  512-wide out column blocks
EN = E * NS                # 49
NP_FP8 = np.dtype(ml_dtypes.float8_e4m3)
NP_BF16 = np.dtype(ml_dtypes.bfloat16)

FK1 = 2                    # fp8 k-tiles in layer 1 (of KT1)
FK2 = 4                    # fp8 k-tiles in layer 2 (of KT2)
BK1 = KT1 - FK1            # bf16 k-tiles, layer 1
BK2 = KT2 - FK2            # bf16 k-tiles, layer 2

# power-of-two scales keeping e4m3 operands in the normal range
SX = 16.0      # x * SX      (x ~ N(0,1))
SW1 = 128.0    # W1 * SW1    (W1 ~ N(0, 1/1024))
SH = 16.0      # h * SH      (h = relu(~N(0,1)))
SW2 = 128.0    # W2 * SW2    (W2 ~ N(0, 1/2048))
L1_EVAC_SCALE = SH / (SX * SW1)      # psum*this = h*SH
GATE_SCALE = 1.0 / (SH * SW2)        # folded into the gate tile


def _emit_kernel(tc, aps, has_b1, has_b2):
    nc = tc.nc
    ctx = ExitStack()
    with ctx:
        # Pool stack order matters: the expert-weight pools are allocated
        # BEFORE the routing pool so they never reuse the routing pool's
        # released SBUF addresses — otherwise Tile serializes the first
        # weight DMAs behind every routing matmul (measured 13µs PE stall).
        consts = ctx.enter_context(tc.tile_pool(name="consts", bufs=1))
        w1pool = ctx.enter_context(tc.tile_pool(name="w1", bufs=2))
        w2pool = ctx.enter_context(tc.tile_pool(name="w2", bufs=2))
        htpool = ctx.enter_context(tc.tile_pool(name="ht", bufs=1))
        tmppool = ctx.enter_context(tc.tile_pool(name="tmp", bufs=3))
        l1ps = ctx.enter_context(tc.tile_pool(name="l1ps", bufs=4, space="PSUM"))
        l2ps = ctx.enter_context(tc.tile_pool(name="l2ps", bufs=4, space="PSUM"))
        rpool = tc.alloc_tile_pool(name="routing", bufs=1)

        # ---- PE warm-up: dummy matmuls from memset tiles (no input deps)
        # fill the ~10µs DMA boot window and flip the HAM clock gate to 8/8
        # before the real matmuls run. ------------------------------------
        warm_sb = rpool.tile([128, 512], BF16)
        nc.vector.memset(warm_sb[:, :], 0.0)
        warm_ps = l1ps.tile([128, 512], F32, tag="ps1", name="warm_ps")
        for _ in range(32):
            nc.tensor.matmul(
                warm_ps[:, :], lhsT=warm_sb[:, 0:128], rhs=warm_sb[:, :],
                start=True, stop=True,
            )

        # ---- layer-1 inputs lead the sync queue so expert 0 starts as
        # early as possible; routing inputs (xT fp32 + S tables) follow. ---
        sflat_sb = rpool.tile([128, KT1, EN], F32)
        xt8_sb = consts.tile([128, FK1, BL], FP8)
        nc.sync.dma_start(xt8_sb[:, :, :], aps["xT8"].rearrange("(t p) b -> p t b", p=128))
        xtb_sb = consts.tile([128, BK1, BL], BF16)
        nc.sync.dma_start(xtb_sb[:, :, :], aps["xTb"].rearrange("(t p) b -> p t b", p=128))

        gate_sb = consts.tile([128, NB, E], F32)
        acc_sb = consts.tile([128, NB, H2], F32)
        if has_b1:
            b1_sb = consts.tile([128, E * MT1], F32)
            nc.sync.dma_start(b1_sb[:, :], aps["b1t"][:, :])
        if has_b2:
            b2_sb = consts.tile([1, E * H2], BF16)
            nc.sync.dma_start(b2_sb[:, :], aps["b2f"][:, :])
            ones_sb = consts.tile([1, 128], BF16)
            nc.vector.memset(ones_sb[:, :], 1.0)

        def dma_weights(e):
            """Queue expert e's weights; returns (w1f, w1b, w2f, w2b) tiles.

            All bulk traffic rides the sync HWDGE queue in program order —
            the per-core DMA fabric saturates at ~350GB/s regardless of
            queue count, so ordering (not parallel queues) is what matters.
            Two half-DMAs per bf16 weight: one trigger splits across all 16
            SDMA engines, and halves complete earlier than one monolithic
            semaphore.
            """
            w1f_sb = w1pool.tile([128, FK1, H1], FP8, tag="w1f")
            nc.sync.dma_start(
                w1f_sb[:, :, :], aps["w1f"][e].rearrange("(t p) h -> p t h", p=128)
            )
            w1b_sb = w1pool.tile([128, BK1, H1], BF16, tag="w1b")
            w1b_src = aps["w1b"][e].rearrange("(t p) h -> p t h", p=128)
            # Column-split halves: layer 1's m-loop consumes columns in
            # order, so m-tiles 0-7 start as soon as the first half lands.
            nc.sync.dma_start(w1b_sb[:, :, 0 : H1 // 2], w1b_src[:, :, 0 : H1 // 2])
            nc.sync.dma_start(w1b_sb[:, :, H1 // 2 :], w1b_src[:, :, H1 // 2 :])
            w2f_sb = w2pool.tile([128, FK2, H2], FP8, tag="w2f")
            nc.sync.dma_start(
                w2f_sb[:, :, :], aps["w2f"][e].rearrange("(t p) o -> p t o", p=128)
            )
            w2b_sb = w2pool.tile([128, BK2, H2], BF16, tag="w2b")
            w2b_src = aps["w2b"][e].rearrange("(t p) o -> p t o", p=128)
            # Column-split halves: the (mb, no=0) PSUM groups only need the
            # first output-column half, so layer 2 can start 1.5MB earlier.
            nc.sync.dma_start(w2b_sb[:, :, 0 : H2 // 2], w2b_src[:, :, 0 : H2 // 2])
            nc.sync.dma_start(w2b_sb[:, :, H2 // 2 :], w2b_src[:, :, H2 // 2 :])
            return w1f_sb, w1b_sb, w2f_sb, w2b_sb

        def layer1(e, w1f_sb, w1b_sb, interleave=None):
            """hT[f, b] = relu(sum_d W1[d, f]*x[b, d] + b1[f]), mixed fp8/bf16.

            Returns (ht8, htb): m-tiles < FK2 evacuate to fp8 (they are
            layer 2's fp8 k-range), the rest to bf16.

            interleave: {m: emit_fn} — expert 0 slips the routing matmuls
            between its late m-groups, keeping the PE stream dense so the
            49-wide fp32 matmuls never let HAM clock-gate the array, and
            the gate is ready right when layer 2's evacuations want it.
            """
            ht8_sb = htpool.tile([128, FK2, BL], FP8, tag="ht8")
            htb_sb = htpool.tile([128, BK2, BL], BF16, tag="htb")
            for m in range(MT1):
                if interleave and m in interleave:
                    interleave[m]()
                ps = l1ps.tile([128, BL], F32, tag="ps1")
                for j in range(FK1 // 2):
                    nc.tensor.matmul(
                        ps[:, :],
                        lhsT=w1f_sb[:, 2 * j : 2 * j + 2, bass.ts(m, 128)],
                        rhs=xt8_sb[:, 2 * j : 2 * j + 2, :],
                        start=(j == 0), stop=False,
                        perf_mode=DR, skip_group_check=True,
                    )
                for kt in range(BK1):
                    nc.tensor.matmul(
                        ps[:, :],
                        lhsT=w1b_sb[:, kt, bass.ts(m, 128)],
                        rhs=xtb_sb[:, kt, :],
                        start=False, stop=(kt == BK1 - 1),
                        skip_group_check=True,
                    )
                dst = ht8_sb[:, m, :] if m < FK2 else htb_sb[:, m - FK2, :]
                bias1 = b1_sb[:, e * MT1 + m : e * MT1 + m + 1] if has_b1 else 0.0
                # Evacuations alternate scalar/vector so neither engine's
                # FIFO becomes the PSUM-slot bottleneck.
                # (The vector path has no bias input, so b1 forces scalar.)
                if has_b1 or m % 2 == 0:
                    nc.scalar.activation(
                        dst, ps[:, :], AF.Relu, bias=bias1, scale=L1_EVAC_SCALE
                    )
                else:
                    nc.vector.tensor_scalar(
                        out=dst, in0=ps[:, :],
                        scalar1=L1_EVAC_SCALE, scalar2=0.0,
                        op0=ALU.mult, op1=ALU.max,
                    )
            return ht8_sb, htb_sb

        def layer2(e, ht8_sb, htb_sb, w2f_sb, w2b_sb):
            """out[b, o] += gate[b, e]*relu(sum_h hT[h, b]*W2[h, o] + b2[o])."""
            for mb in range(NB):
                for no in range(NO):
                    ps2 = l2ps.tile([128, 512], F32, tag="ps2")
                    for j in range(FK2 // 2):
                        nc.tensor.matmul(
                            ps2[:, :],
                            lhsT=ht8_sb[:, 2 * j : 2 * j + 2, bass.ts(mb, 128)],
                            rhs=w2f_sb[:, 2 * j : 2 * j + 2, bass.ts(no, 512)],
                            start=(j == 0), stop=False,
                            perf_mode=DR, skip_group_check=True,
                        )
                    for kt in range(BK2):
                        nc.tensor.matmul(
                            ps2[:, :],
                            lhsT=htb_sb[:, kt, bass.ts(mb, 128)],
                            rhs=w2b_sb[:, kt, bass.ts(no, 512)],
                            start=False,
                            stop=(kt == BK2 - 1 and not has_b2),
                            skip_group_check=True,
                        )
                    if has_b2:
                        nc.tensor.matmul(
                            ps2[:, :],
                            lhsT=ones_sb[:, :],
                            rhs=b2_sb[:, e * H2 + no * 512 : e * H2 + (no + 1) * 512],
                            start=False, stop=True, skip_group_check=True,
                        )
                    gcol = gate_sb[:, mb, e : e + 1]
                    if e == 0:
                        nc.scalar.activation(
                            acc_sb[:, mb, bass.ts(no, 512)], ps2[:, :], AF.Relu, scale=gcol
                        )
                    else:
                        tmp = tmppool.tile([128, 512], F32, tag="tmp")
                        nc.scalar.activation(tmp[:, :], ps2[:, :], AF.Relu, scale=gcol)
                        nc.vector.tensor_tensor(
                            out=acc_sb[:, mb, bass.ts(no, 512)],
                            in0=acc_sb[:, mb, bass.ts(no, 512)],
                            in1=tmp[:, :], op=ALU.add,
                        )
                    # Per-half-tile output DMA so the store overlaps the
                    # remaining compute instead of tailing the kernel.
                    if e == E - 1:
                        nc.sync.dma_start(
                            aps["out"].rearrange("(t p) o -> p t o", p=128)[
                                :, mb, no * 512 : (no + 1) * 512
                            ],
                            acc_sb[:, mb, bass.ts(no, 512)],
                        )

        # ---- expert 0 layer 1 first: the PE's first real work only needs
        # x + W1[0], which lead the DMA queue. -----------------------------
        w1f0_sb = w1pool.tile([128, FK1, H1], FP8, tag="w1f")
        nc.sync.dma_start(
            w1f0_sb[:, :, :], aps["w1f"][0].rearrange("(t p) h -> p t h", p=128)
        )
        # Expert 0's bf16 W1 streams in four column-quarters: layer 1's
        # m-tiles 0-3 only need the first 0.75MB, so the PE starts ~2µs
        # earlier than with halves. sflat (routing, consumed at m>=10)
        # rides after the second quarter.
        w1b0_sb = w1pool.tile([128, BK1, H1], BF16, tag="w1b")
        w1b0_src = aps["w1b"][0].rearrange("(t p) h -> p t h", p=128)
        for qq in range(2):
            nc.sync.dma_start(
                w1b0_sb[:, :, qq * 512 : (qq + 1) * 512],
                w1b0_src[:, :, qq * 512 : (qq + 1) * 512],
            )
        nc.sync.dma_start(sflat_sb[:, :, :], aps["sflat"].rearrange("(t p) j -> p t j", p=128))
        for qq in range(2, 4):
            nc.sync.dma_start(
                w1b0_sb[:, :, qq * 512 : (qq + 1) * 512],
                w1b0_src[:, :, qq * 512 : (qq + 1) * 512],
            )

        # ---- routing inputs: xT quarters (kt-major matmuls consume k-slices
        # as they land), then the small tables, then expert 0's W2. --------
        xt_sb = rpool.tile([128, KT1, BL], F32)
        xt_src = aps["xT"].rearrange("(t p) b -> p t b", p=128)
        for qq in range(4):
            nc.sync.dma_start(
                xt_sb[:, 2 * qq : 2 * qq + 2, :], xt_src[:, 2 * qq : 2 * qq + 2, :]
            )
        sett_sb = rpool.tile([10, EN], F32)
        nc.sync.dma_start(sett_sb[:, :], aps["sett"][:, :])
        scolr_sb = rpool.tile([128, NB * EN], F32)
        nc.sync.dma_start(scolr_sb[:, :], aps["scol_rep"][:, :])
        srow10_sb = rpool.tile([10, BL], F32)
        nc.sync.dma_start(srow10_sb[:, :], aps["srow"].to_broadcast((10, BL)))
        io7_sb = rpool.tile([128, NB * EN], F32)
        nc.sync.dma_start(io7_sb[:, :], aps["iota7"].to_broadcast((128, NB * EN)))
        io10_sb = rpool.tile([10, 1], F32)
        nc.sync.dma_start(io10_sb[:, :], aps["iota10"][:, :])

        w2f0_sb = w2pool.tile([128, FK2, H2], FP8, tag="w2f")
        nc.sync.dma_start(
            w2f0_sb[:, :, :], aps["w2f"][0].rearrange("(t p) o -> p t o", p=128)
        )
        w2b0_sb = w2pool.tile([128, BK2, H2], BF16, tag="w2b")
        w2b0_src = aps["w2b"][0].rearrange("(t p) o -> p t o", p=128)
        nc.sync.dma_start(w2b0_sb[:, :, 0 : H2 // 2], w2b0_src[:, :, 0 : H2 // 2])
        nc.sync.dma_start(w2b0_sb[:, :, H2 // 2 :], w2b0_src[:, :, H2 // 2 :])

        # onehot over embedding rows, [10, BL]: onehot[r, b] = (scene[b] == r).
        # On GPSIMD (otherwise idle): the DVE FIFO is full of layer-1
        # evacuations, which would delay this until ~the routing tail and
        # leave the PE's onehot matmuls briefly starved (measured 6.8µs
        # half-clock dip at the L2(0) transition).
        onehot_sb = rpool.tile([10, BL], F32)
        nc.gpsimd.tensor_scalar(
            out=onehot_sb[:, :], in0=srow10_sb[:, :],
            scalar1=io10_sb[:, 0:1], scalar2=None, op0=ALU.is_equal,
        )

        # Routing matmuls slip between expert 0's late L1 m-groups (the xT
        # quarters have landed by then), keeping the PE stream dense so the
        # 49-wide fp32 matmuls never let HAM clock-gate the array, and the
        # gate is ready right when layer 2's evacuations want it. The psr
        # PSUM groups live in the L2 pool: its first real tiles are only
        # allocated after the gp copies release these, so the rotation
        # cannot deadlock against L1's.
        psr = []
        for t in range(NB):
            psr_t = l2ps.tile([128, EN], F32, tag="ps2", name=f"psr{t}")
            psr.append(psr_t)
        gp = rpool.tile([128, NB * EN], F32)  # all 4 b-tiles side by side

        def routing_chunk(kts):
            def emit():
                for kt in kts:
                    for t in range(NB):
                        nc.tensor.matmul(
                            psr[t][:, :],
                            lhsT=xt_sb[:, kt, bass.ts(t, 128)],
                            rhs=sflat_sb[:, kt, :],
                            start=(kt == 0), stop=False,
                        )
            return emit

        def routing_finish():
            for t in range(NB):
                nc.tensor.matmul(
                    psr[t][:, :],
                    lhsT=onehot_sb[:, bass.ts(t, 128)],
                    rhs=sett_sb[:, :],
                    start=False, stop=True,
                )

        ht8_0, htb_0 = layer1(
            0, w1f0_sb, w1b0_sb,
            interleave={
                10: routing_chunk([0, 1]),
                11: routing_chunk([2, 3]),
                12: routing_chunk([4, 5]),
                13: routing_chunk([6, 7]),
                14: routing_finish,
            },
        )
        for t in range(NB):
            nc.scalar.copy(gp[:, bass.ts(t, EN)], psr[t][:, :])

        def routing_chain():
            """Gate computation, fused over all 4 b-tiles ([128, 4*49]).

            Emitted AFTER layer 1 of expert 0: the scalar engine's queue is
            strict FIFO, so emitting this serial chain before the L1 PSUM
            evacuations would block them (and stall the PE on PSUM slots).
            The gate is only consumed by expert 0's layer-2 evacuation.
            """
            NE = NB * E  # 28
            gp4 = gp.rearrange("p (t e s) -> p (t e) s", s=NS, e=E)
            eex = rpool.tile([128, NB * EN], F32)
            nc.scalar.activation(eex[:, :], gp[:, :], AF.Exp)
            z = rpool.tile([128, NE], F32)
            nc.vector.tensor_reduce(out=z[:, :], in_=eex.rearrange("p (t e s) -> p (t e) s", s=NS, e=E), axis=AX.X, op=ALU.add)
            logz = rpool.tile([128, NE], F32)
            nc.scalar.activation(logz[:, :], z[:, :], AF.Ln)
            sg = rpool.tile([128, NE], F32)
            nc.vector.tensor_reduce(out=sg[:, :], in_=gp4, axis=AX.X, op=ALU.add)
            q = rpool.tile([128, NE], F32)
            nc.vector.scalar_tensor_tensor(
                out=q[:, :], in0=sg[:, :], scalar=1.0 / NS, in1=logz[:, :],
                op0=ALU.mult, op1=ALU.subtract,
            )
            oh = rpool.tile([128, NB * EN], F32)
            nc.vector.tensor_tensor(out=oh[:, :], in0=io7_sb[:, :], in1=scolr_sb[:, :], op=ALU.is_equal)
            gsel = rpool.tile([128, NB * EN], F32)
            nc.vector.tensor_tensor(out=gsel[:, :], in0=gp[:, :], in1=oh[:, :], op=ALU.mult)
            s1s = rpool.tile([128, NE], F32)
            nc.vector.tensor_reduce(out=s1s[:, :], in_=gsel.rearrange("p (t e s) -> p (t e) s", s=NS, e=E), axis=AX.X, op=ALU.add)
            score1 = rpool.tile([128, NE], F32)
            nc.vector.tensor_tensor(out=score1[:, :], in0=s1s[:, :], in1=logz[:, :], op=ALU.subtract)

            lg = rpool.tile([128, NE], F32)
            nc.scalar.activation(lg[:, :], score1[:, :], AF.Exp)     # G at scene, in (0,1)
            el = rpool.tile([128, NE], F32)
            nc.scalar.activation(el[:, :], lg[:, :], AF.Exp)         # softmax numerator
            # per-b-tile scalars ([128,1]) for the reductions' broadcasts
            ssum = rpool.tile([128, NB], F32)
            rs = rpool.tile([128, NB], F32)
            m1 = rpool.tile([128, NB], F32)
            m2 = rpool.tile([128, NB], F32)
            k1 = rpool.tile([128, NE], F32)
            k2 = rpool.tile([128, NE], F32)
            g0 = rpool.tile([128, NE], F32)
            el3 = el.rearrange("p (t e) -> p t e", e=E)
            sc3 = score1.rearrange("p (t e) -> p t e", e=E)
            q3 = q.rearrange("p (t e) -> p t e", e=E)
            nc.vector.tensor_reduce(out=ssum[:, :], in_=el3, axis=AX.X, op=ALU.add)
            nc.vector.reciprocal(rs[:, :], ssum[:, :])
            nc.vector.tensor_reduce(out=m1[:, :], in_=sc3, axis=AX.X, op=ALU.min)
            nc.vector.tensor_reduce(out=m2[:, :], in_=q3, axis=AX.X, op=ALU.min)
            for t in range(NB):
                nc.vector.tensor_scalar(
                    out=k1[:, bass.ts(t, E)], in0=score1[:, bass.ts(t, E)],
                    scalar1=m1[:, t : t + 1], scalar2=None, op0=ALU.is_equal,
                )
                nc.vector.tensor_scalar(
                    out=k2[:, bass.ts(t, E)], in0=q[:, bass.ts(t, E)],
                    scalar1=m2[:, t : t + 1], scalar2=None, op0=ALU.is_equal,
                )
                nc.vector.tensor_scalar(
                    out=g0[:, bass.ts(t, E)], in0=el[:, bass.ts(t, E)],
                    scalar1=rs[:, t : t + 1], scalar2=None, op0=ALU.mult,
                )
            kill = rpool.tile([128, NE], F32)
            nc.vector.tensor_tensor(out=kill[:, :], in0=k1[:, :], in1=k2[:, :], op=ALU.mult)
            sel = rpool.tile([128, NE], F32)
            nc.vector.tensor_scalar(
                out=sel[:, :], in0=kill[:, :], scalar1=-1.0, scalar2=1.0,
                op0=ALU.mult, op1=ALU.add,
            )
            gate_flat = gate_sb.rearrange("p t e -> p (t e)")
            # gate = g0 * sel, pre-scaled by 1/(SH*SW2) for the L2 evacuation
            nc.vector.scalar_tensor_tensor(
                out=gate_flat[:, :], in0=g0[:, :], scalar=GATE_SCALE,
                in1=sel[:, :], op0=ALU.mult, op1=ALU.mult,
            )

        routing_chain()
        rpool.release()

        # ---- expert 0 layer 2, then experts 1..6 -------------------------
        layer2(0, ht8_0, htb_0, w2f0_sb, w2b0_sb)
        for e in range(1, E):
            w1f_sb, w1b_sb, w2f_sb, w2b_sb = dma_weights(e)
            ht8_sb, htb_sb = layer1(e, w1f_sb, w1b_sb)
            layer2(e, ht8_sb, htb_sb, w2f_sb, w2b_sb)


def build(has_b1, has_b2):
    """Build + schedule + compile the Bass program. Returns nc."""
    nc = bacc.Bacc("TRN2", target_bir_lowering=False, debug=False)
    aps = {}
    aps["xT"] = nc.dram_tensor("xT", [D, BL], F32, kind="ExternalInput").ap()
    aps["xT8"] = nc.dram_tensor("xT8", [FK1 * 128, BL], FP8, kind="ExternalInput").ap()
    aps["xTb"] = nc.dram_tensor("xTb", [BK1 * 128, BL], BF16, kind="ExternalInput").ap()
    aps["w1f"] = nc.dram_tensor("w1f", [E, FK1 * 128, H1], FP8, kind="ExternalInput").ap()
    aps["w1b"] = nc.dram_tensor("w1b", [E, BK1 * 128, H1], BF16, kind="ExternalInput").ap()
    aps["w2f"] = nc.dram_tensor("w2f", [E, FK2 * 128, H2], FP8, kind="ExternalInput").ap()
    aps["w2b"] = nc.dram_tensor("w2b", [E, BK2 * 128, H2], BF16, kind="ExternalInput").ap()
    if has_b1:
        aps["b1t"] = nc.dram_tensor("b1t", [128, E * MT1], F32, kind="ExternalInput").ap()
    if has_b2:
        aps["b2f"] = nc.dram_tensor("b2f", [1, E * H2], BF16, kind="ExternalInput").ap()
    aps["sflat"] = nc.dram_tensor("sflat", [D, EN], F32, kind="ExternalInput").ap()
    aps["sett"] = nc.dram_tensor("sett", [10, EN], F32, kind="ExternalInput").ap()
    aps["scol_rep"] = nc.dram_tensor("scol_rep", [128, NB * EN], F32, kind="ExternalInput").ap()
    aps["srow"] = nc.dram_tensor("srow", [1, BL], F32, kind="ExternalInput").ap()
    aps["iota7"] = nc.dram_tensor("iota7", [1, NB * EN], F32, kind="ExternalInput").ap()
    aps["iota10"] = nc.dram_tensor("iota10", [10, 1], F32, kind="ExternalInput").ap()
    aps["out"] = nc.dram_tensor("out", [BL, H2], F32, kind="ExternalOutput").ap()

    with tile.TileContext(nc) as tc:
        _emit_kernel(tc, aps, has_b1, has_b2)
    nc.compile()
    return nc


def make_in_maps(inputs):
    """Host-side layout prep + batch sharding. Returns (in_maps, has_b1, has_b2)."""
    x = np.ascontiguousarray(np.asarray(inputs["x"], dtype=np.float32))
    scene = np.asarray(inputs["scene"]).astype(np.int64)
    W1 = np.asarray(inputs["W1"], dtype=np.float32)
    b1 = np.asarray(inputs["b1"], dtype=np.float32)
    W2 = np.asarray(inputs["W2"], dtype=np.float32)
    b2 = np.asarray(inputs["b2"], dtype=np.float32)
    S = np.asarray(inputs["S"], dtype=np.float32)
    scene_emb = np.asarray(inputs["scene_emb"], dtype=np.float32)

    has_b1 = bool(np.any(b1))
    has_b2 = bool(np.any(b2))

    d_f1 = FK1 * 128           # fp8 rows of the L1 contraction
    h_f2 = FK2 * 128           # fp8 rows of the L2 contraction
    w1s = W1 * SW1
    w2s = W2 * SW2
    shared = {
        "w1f": np.ascontiguousarray(w1s[:, :d_f1, :].astype(NP_FP8)),
        "w1b": np.ascontiguousarray(w1s[:, d_f1:, :].astype(NP_BF16)),
        "w2f": np.ascontiguousarray(w2s[:, :h_f2, :].astype(NP_FP8)),
        "w2b": np.ascontiguousarray(w2s[:, h_f2:, :].astype(NP_BF16)),
        "sflat": np.ascontiguousarray(S[:, :D, :].transpose(1, 2, 0).reshape(D, EN)),
        "sett": np.ascontiguousarray(
            np.einsum("rm,sme->res", scene_emb, S[:, D:, :]).reshape(scene_emb.shape[0], EN)
        ),
        "iota7": np.tile(np.arange(EN, dtype=np.float32) % NS, NB).reshape(1, NB * EN),
        "iota10": np.arange(10, dtype=np.float32).reshape(10, 1),
    }
    if has_b1:
        shared["b1t"] = np.ascontiguousarray(
            b1.reshape(E, MT1, 128).transpose(2, 0, 1).reshape(128, E * MT1)
            * SH  # activation computes relu(psum*scale + bias); bias = b1*SH
        )
    if has_b2:
        shared["b2f"] = np.ascontiguousarray(
            (b2 * SH * SW2).astype(NP_BF16).reshape(1, E * H2)
        )

    in_maps = []
    for c in range(N_CORES):
        xs = x[c * BL : (c + 1) * BL]
        sc = scene[c * BL : (c + 1) * BL]
        xT = np.ascontiguousarray(xs.T)
        xTs = xT * SX
        m = dict(shared)
        m["xT"] = xT
        m["xT8"] = np.ascontiguousarray(xTs[:d_f1].astype(NP_FP8))
        m["xTb"] = np.ascontiguousarray(xTs[d_f1:].astype(NP_BF16))
        scol = sc.reshape(NB, 128).T.astype(np.float32)          # [128, NB]
        m["scol_rep"] = np.ascontiguousarray(
            np.repeat(scol[:, :, None], EN, axis=2).reshape(128, NB * EN)
        )
        m["srow"] = np.ascontiguousarray(sc.astype(np.float32).reshape(1, BL))
        in_maps.append(m)
    return in_maps, has_b1, has_b2


_NC_CACHE = {}


def get_compiled(has_b1, has_b2):
    key = (has_b1, has_b2)
    if key not in _NC_CACHE:
        _NC_CACHE[key] = build(has_b1, has_b2)
    return _NC_CACHE[key]


def run(inputs, trace=False, **kwargs):
    """Run on hardware; returns (full_output, BassKernelResults)."""
    in_maps, has_b1, has_b2 = make_in_maps(inputs)
    nc = get_compiled(has_b1, has_b2)
    res = run_bass_kernel_spmd(nc, in_maps, core_ids=list(range(N_CORES)), trace=trace, **kwargs)
    parts = [res.results[c]["out"] for c in range(N_CORES)]
    out = np.concatenate(parts, axis=0).astype(np.float32)
    full = np.ascontiguousarray(np.broadcast_to(out[None], (T, B, H2)))
    return full, res


def kernel(**inputs):
    full, _ = run(inputs, trace=False)
    return full
